# revision 6
# baseline (speedup 1.0000x reference)
"""Trainium2 Bass kernel for multi-head causal attention (nn_MultiHeadAttention).

Full-model shapes: x [4, 2048, 1024], 16 heads x 64 head-size, Wo [1024, 1024].

Sharding (8 cores): shard = (batch b, head-group g of 8 heads); core = 2*b + g.
Each core computes, for its batch and its 8 heads:
  QT/KT [hs, T] (head pairs packed into 128 partitions) and VA = [V | 1] [T, 65],
  ST = K @ Q^T blocks [s-part, t-free] (causal blocks only, band narrowed),
  expST = exp(ST/8), diagonal 128x128 sub-block masked post-exp with a 0/1 tri,
  OT = [V | 1]^T @ expST  -> rows 0:64 unnormalized output (transposed),
                             row 64 the softmax denominator l(t),
  concatT = OT[0:64] * (1/l) broadcast,
  y_partial = concatT^T @ Wo[512*g : 512*(g+1)]  [T, 1024]  (stored bf16).
Host sums the two head-group partials per batch and adds the bias.

Head pairs share one [128,1024] ST psum tile (h0 -> cols 0:512, h1 -> 512:1024,
PE row groups 0:63 / 64:127) so a single strided ACTIVATE computes exp for
both heads. Softmax needs no max-subtraction: scores are q.k/8 with |q|,|k|
~ 0.6, so exp() stays in a tiny range and matches jax.nn.softmax to fp32
rounding.

Scheduling: scalar ACT (exp) is the per-chunk pacing engine (~1.1us for
[128,1024]); projection / output matmuls are drained as fine-grained filler
units (1-2 matmuls) between attention chunks so the PE stays dense without
starving ACT.  K/Q projection groups accumulate two j-tiles per weight load
so walrus dedups the LDWEIGHTS of the matmul pair.
"""

import os
from contextlib import ExitStack

import numpy as np
import ml_dtypes

N_HEADS = 16
HEAD_SIZE = 64
N_EMBED = 1024
B, T = 4, 2048
P = 128
NE = N_EMBED // P          # 8 e-chunks
NT5 = T // 512             # 4 t-tiles of 512
NT1 = T // P               # 16 t-blocks of 128
NH = N_HEADS // 2          # 8 heads per core
NPAIR = NH // 2            # 4 head pairs per core
DGRP = NH * HEAD_SIZE      # 512 concat rows per core

# matmul dtype: "bf16" or "f32r" (fp32 data, relaxed-precision PE mode)
MM_DT = os.environ.get("KERNEL_MM_DT", "bf16")

_CACHED_NC = {}


def _build_bass(mm_dt_name: str):
    import concourse.bass as bass  # noqa: F401
    import concourse.tile as tile
    from concourse import bacc, mybir

    f32 = mybir.dt.float32
    if mm_dt_name == "bf16":
        dt_mm = mybir.dt.bfloat16
        mm_cast = lambda ap: ap  # noqa: E731
    else:
        dt_mm = f32
        mm_cast = lambda ap: ap.bitcast(mybir.dt.float32r)  # noqa: E731
    Exp = mybir.ActivationFunctionType.Exp

    nc = bacc.Bacc("TRN2", target_bir_lowering=False, debug=False, num_devices=8)

    xT_d = nc.dram_tensor("xT", [N_EMBED, T], dt_mm, kind="ExternalInput")
    wq_d = nc.dram_tensor("wq", [N_EMBED, DGRP], dt_mm, kind="ExternalInput")
    wk_d = nc.dram_tensor("wk", [N_EMBED, DGRP], dt_mm, kind="ExternalInput")
    wv_d = nc.dram_tensor("wv", [N_EMBED, DGRP], dt_mm, kind="ExternalInput")
    wo_d = nc.dram_tensor("wo", [DGRP, N_EMBED], dt_mm, kind="ExternalInput")
    trib_d = nc.dram_tensor("trib", [P, P], dt_mm, kind="ExternalInput")
    y_d = nc.dram_tensor("y", [T, N_EMBED], dt_mm, kind="ExternalOutput")

    xT_ap = xT_d.ap().rearrange("(o p) t -> p o t", p=P)    # [128, 8, 2048]
    wq_ap = wq_d.ap().rearrange("(o p) m -> p o m", p=P)    # [128, 8, 512]
    wk_ap = wk_d.ap().rearrange("(o p) m -> p o m", p=P)
    wv_ap = wv_d.ap().rearrange("(o p) m -> p o m", p=P)
    wo_ap = wo_d.ap().rearrange("(o p) e -> p o e", p=P)    # [128, 4, 1024]
    y_ap = y_d.ap().rearrange("(o p) e -> p o e", p=P)      # [128, 16, 1024]

    with tile.TileContext(nc) as tc, ExitStack() as ctx:
        const = ctx.enter_context(tc.tile_pool(name="const", bufs=1))
        persist = ctx.enter_context(tc.tile_pool(name="persist", bufs=1))
        # PSUM: pb1 4x1 banks (filler psums + OT tiles) + stp 2x2 banks = 8.
        pb1 = ctx.enter_context(tc.tile_pool(name="pb1", bufs=4, space="PSUM"))
        stp = ctx.enter_context(tc.tile_pool(name="stp", bufs=2, space="PSUM"))
        expool = ctx.enter_context(tc.tile_pool(name="expool", bufs=6))
        rp = ctx.enter_context(tc.tile_pool(name="rp", bufs=4))
        ysbp = ctx.enter_context(tc.tile_pool(name="ysb", bufs=2))

        trib_sb = const.tile([P, P], dt_mm)
        warm = const.tile([1, 2], f32)

        # persistent tensors (bf16: ~125 KB/partition total incl pools)
        xt_sb = persist.tile([P, NE, T], dt_mm)
        wv_sb = persist.tile([P, NE, DGRP], dt_mm)
        wk_sb = persist.tile([P, NE, DGRP], dt_mm)
        wq_sb = persist.tile([P, NE, DGRP], dt_mm)
        wo_sb = persist.tile([P, NPAIR, N_EMBED], dt_mm)
        VA = persist.tile([P, NT1, NH, HEAD_SIZE + 1], dt_mm)
        CT = persist.tile([P, NPAIR, T], dt_mm)
        QTs = [persist.tile([P, T], dt_mm, name=f"QT_{pp}") for pp in range(NPAIR)]
        KTs = [persist.tile([P, T], dt_mm, name=f"KT_{pp}") for pp in range(NPAIR)]

        # ACT table pre-warm: first exp pays the ~2.7us table load during the
        # initial DMA wait instead of on the first attention chunk.
        nc.vector.memset(warm[:], 0.0)
        nc.scalar.activation(warm[:], warm[:], Exp, scale=1.0)
        nc.vector.memset(VA[:, :, :, HEAD_SIZE : HEAD_SIZE + 1], 1.0)

        # ---- input DMAs, consumption order, large tensors split so the
        # pieces land on parallel queues and the first matmuls start early.
        # Head phase needs: xt (t 0:512 for all e), wv (all), wk/wq pair-0
        # columns.
        nc.sync.dma_start(trib_sb[:], trib_d.ap())
        for e in range(NE):
            nc.sync.dma_start(xt_sb[:, e, 0:512], xT_ap[:, e, 0:512])
            nc.sync.dma_start(wv_sb[:, e, :], wv_ap[:, e, :])
        for h in range(2):
            nc.sync.dma_start(wk_sb[:, 4 * h : 4 * h + 4, 0:P],
                              wk_ap[:, 4 * h : 4 * h + 4, 0:P])
            nc.sync.dma_start(wq_sb[:, 4 * h : 4 * h + 4, 0:P],
                              wq_ap[:, 4 * h : 4 * h + 4, 0:P])
        for tq in range(1, 4):
            for e in range(NE):
                nc.sync.dma_start(xt_sb[:, e, 512 * tq : 512 * (tq + 1)],
                                  xT_ap[:, e, 512 * tq : 512 * (tq + 1)])
        for h in range(2):
            nc.sync.dma_start(wk_sb[:, 4 * h : 4 * h + 4, P:DGRP],
                              wk_ap[:, 4 * h : 4 * h + 4, P:DGRP])
            nc.sync.dma_start(wq_sb[:, 4 * h : 4 * h + 4, P:DGRP],
                              wq_ap[:, 4 * h : 4 * h + 4, P:DGRP])
        for dc in range(NPAIR):
            nc.sync.dma_start(wo_sb[:, dc, :], wo_ap[:, dc, :])

        # ---------------- V projection (one t-block of 128) ----------------
        # stationary = xt chunk, moving = wv; out [t 128, 512] -> VA[:,tb,:,1:]
        def v_units(tb):
            hold = {}

            def mm(e):
                if e == 0:
                    hold["vp"] = pb1.tile([P, DGRP], f32, tag="b1",
                                          name=f"v_ps_{tb}")
                nc.tensor.matmul(
                    hold["vp"][:],
                    mm_cast(xt_sb[:, e, P * tb : P * (tb + 1)]),
                    mm_cast(wv_sb[:, e, :]),
                    start=(e == 0),
                    stop=(e == NE - 1),
                )

            def evict():
                nc.vector.tensor_copy(
                    VA[:, tb, :, 0:HEAD_SIZE],
                    hold["vp"][:].rearrange("p (h d) -> p h d", d=HEAD_SIZE),
                )

            return [lambda e=e: mm(e) for e in range(NE)] + [evict]

        # -------- K/Q projection: two j-tiles per stationary load ---------
        # stationary = w chunk [e 128, pair 128]; for each e the two matmuls
        # (j = 2jj, 2jj+1) share the stationary so walrus dedups the
        # LDWEIGHTS.  Two psum tiles held across the e loop.
        def qk_units(p, which, jj):
            w_sb = wk_sb if which == 0 else wq_sb
            dst = KTs[p] if which == 0 else QTs[p]
            hold = {}

            def mm2(e):
                if e == 0:
                    for ji in range(2):
                        hold[ji] = pb1.tile([P, 512], f32, tag="b1",
                                            name=f"qk_ps_{p}_{which}_{jj}_{ji}")
                for ji in range(2):
                    j = 2 * jj + ji
                    nc.tensor.matmul(
                        hold[ji][:],
                        mm_cast(w_sb[:, e, P * p : P * (p + 1)]),
                        mm_cast(xt_sb[:, e, 512 * j : 512 * (j + 1)]),
                        start=(e == 0),
                        stop=(e == NE - 1),
                    )

            def evict(ji):
                j = 2 * jj + ji
                nc.vector.tensor_copy(dst[:, 512 * j : 512 * (j + 1)],
                                      hold[ji][:])

            return ([lambda e=e: mm2(e) for e in range(NE)]
                    + [lambda ji=ji: evict(ji) for ji in range(2)])

        # ---- output projection for one t-block: y[tb] = CT^T @ Wo-half ----
        # (dc, eh) loop: the eh pair shares the CT stationary (LDW dedup).
        def proj_units(tb):
            hold = {}

            def mm2(dc):
                if dc == 0:
                    hold[0] = pb1.tile([P, 512], f32, tag="b1",
                                       name=f"y_ps_{tb}_0")
                    hold[1] = pb1.tile([P, 512], f32, tag="b1",
                                       name=f"y_ps_{tb}_1")
                for eh in range(2):
                    nc.tensor.matmul(
                        hold[eh][:],
                        mm_cast(CT[:, dc, P * tb : P * (tb + 1)]),
                        mm_cast(wo_sb[:, dc, 512 * eh : 512 * (eh + 1)]),
                        start=(dc == 0),
                        stop=(dc == NPAIR - 1),
                    )

            def evict():
                ysb = ysbp.tile([P, N_EMBED], dt_mm, tag="ysb", name=f"ysb_{tb}")
                nc.vector.tensor_copy(ysb[:, 0:512], hold[0][:])
                nc.vector.tensor_copy(ysb[:, 512:1024], hold[1][:])
                for q in range(2):
                    nc.sync.dma_start(y_ap[:, tb, 512 * q : 512 * (q + 1)],
                                      ysb[:, 512 * q : 512 * (q + 1)])

            return [lambda dc=dc: mm2(dc) for dc in range(NPAIR)] + [evict]

        # ---------------- head phase: V tb0-3, K/Q pair-0 jj=0 ----------------
        for tb in range(4):
            for u in v_units(tb):
                u()
        for which in range(2):
            for u in qk_units(0, which, 0):
                u()

        # ---- filler queue: flat list of units with (pair, j) deadlines ----
        # deadline key = 4*p + j: all units of a group must have run by the
        # END of that attention (p, j).  Units are drained inside the chunk
        # loop, paced evenly against the chunk index, so ACT never starves
        # behind a projection lump.
        fill_units = []

        def add_group(units, deadline):
            for u in units:
                fill_units.append((deadline, u))

        # V tb4-7 needed by p0 j1 chunks; tb8-11 by j2; tb12-15 by j3.
        for tb in range(4, 8):
            add_group(v_units(tb), 1)
        add_group(qk_units(0, 0, 1), 1)   # KT p0 cols 1024:2048 by p0 j1 end
        add_group(qk_units(0, 1, 1), 1)   # QT p0 jj1 (j1-end hoist reads j2)
        for tb in range(8, 12):
            add_group(v_units(tb), 2)
        for tb in range(12, 16):
            add_group(v_units(tb), 3)
        add_group(qk_units(1, 0, 0), 2)   # K/Q p1 jj0 by p0 j3 end (hoist)
        add_group(qk_units(1, 1, 0), 2)
        add_group(qk_units(1, 0, 1), 4 + 1)
        add_group(qk_units(1, 1, 1), 4 + 1)
        add_group(qk_units(2, 0, 0), 4 + 2)
        add_group(qk_units(2, 1, 0), 4 + 2)
        add_group(qk_units(2, 0, 1), 8 + 1)
        add_group(qk_units(2, 1, 1), 8 + 1)
        add_group(qk_units(3, 0, 0), 8 + 2)
        add_group(qk_units(3, 1, 0), 8 + 2)
        add_group(qk_units(3, 0, 1), 12 + 1)
        add_group(qk_units(3, 1, 1), 12 + 1)

        fill_pos = [0]

        def drain_to(target):
            while fill_pos[0] < min(target, len(fill_units)):
                fill_units[fill_pos[0]][1]()
                fill_pos[0] += 1

        def deadline_target(key):
            # index just past the last unit with deadline <= key
            t = fill_pos[0]
            for i in range(fill_pos[0], len(fill_units)):
                if fill_units[i][0] <= key:
                    t = i + 1
            return t

        # ------- attention -------
        pre_es = {}

        def st_exp(p, j, c):
            KTp, QTp = KTs[p], QTs[p]
            off = P * max(0, c - 4 * j)
            stq = stp.tile([P, 1024], f32, tag="st", name=f"st_{p}_{j}_{c}")
            for hh in range(2):
                nc.tensor.matmul(
                    stq[:, 512 * hh + off : 512 * hh + 512],
                    mm_cast(KTp[64 * hh : 64 * hh + 64, P * c : P * (c + 1)]),
                    mm_cast(
                        QTp[64 * hh : 64 * hh + 64,
                            512 * j + off : 512 * (j + 1)]
                    ),
                    start=True,
                    stop=True,
                )
            stv = stq[:].rearrange("p (g t) -> p g t", g=2)
            es = expool.tile([P, 1024], dt_mm, tag="es",
                             name=f"es_{p}_{j}_{c}")
            esv = es[:].rearrange("p (g t) -> p g t", g=2)
            nc.scalar.activation(
                esv[:, :, off:512], stv[:, :, off:512], Exp, scale=0.125
            )
            if c >= 4 * j:  # diagonal sub-block: zero the upper triangle
                dv = esv[:, :, off : off + P]
                nc.vector.tensor_mul(
                    dv, dv, trib_sb[:, None, :].to_broadcast((P, 2, P))
                )
            return es

        global_chunk = [0]
        TOTAL_CHUNKS = sum(4 * j + 4 for j in range(NT5)) * NPAIR  # 160

        for p in range(NPAIR):
            for j in range(NT5):
                key = 4 * p + j
                ots = [
                    pb1.tile([HEAD_SIZE + 1, 512], f32, tag="b1",
                             name=f"ot_{p}_{j}_{hh}")
                    for hh in range(2)
                ]
                ncs = 4 * j + 4
                # pacing: by chunk c, reach the global even-spread target,
                # and by the last chunk the deadline target for this (p, j)
                dl_target = deadline_target(key)
                for c in range(ncs):
                    off = P * max(0, c - 4 * j)   # band narrowing
                    if (p, j, c) in pre_es:
                        es = pre_es.pop((p, j, c))
                    else:
                        es = st_exp(p, j, c)
                    if c == ncs - 1:
                        # hoist the next block's first two ST+exp ahead of
                        # the last PVs so ACT is never starved across the
                        # boundary
                        nxt = (p, j + 1) if j + 1 < NT5 else (p + 1, 0)
                        if nxt[0] < NPAIR:
                            pre_es[(nxt[0], nxt[1], 0)] = st_exp(nxt[0], nxt[1], 0)
                            pre_es[(nxt[0], nxt[1], 1)] = st_exp(nxt[0], nxt[1], 1)
                    for hh in range(2):
                        nc.tensor.matmul(
                            ots[hh][:, off:512],
                            mm_cast(VA[:, c, 2 * p + hh, :]),
                            mm_cast(es[:, 512 * hh + off : 512 * hh + 512]),
                            start=(c == 0),
                            stop=(c == ncs - 1),
                        )
                    global_chunk[0] += 1
                    g_target = (len(fill_units) * global_chunk[0]
                                + TOTAL_CHUNKS - 1) // TOTAL_CHUNKS
                    tgt = g_target
                    # deadline clamp: spread this j's mandatory units evenly
                    need = dl_target - fill_pos[0]
                    if need > 0:
                        tgt = max(tgt, fill_pos[0]
                                  + (need * (c + 1) + ncs - 1) // ncs)
                    drain_to(tgt)
                for hh in range(2):
                    # OT rows 0:64 unnormalized output, row 64 is l(t).
                    # l staged to partition 0: reciprocal_approx_fast reads
                    # garbage when its input AP starts at partition 64.
                    l0 = rp.tile([1, 512], f32, tag="lrow", name=f"l0_{p}_{j}_{hh}")
                    nc.vector.tensor_copy(l0[:], ots[hh][64:65, :])
                    r = rp.tile([1, 512], f32, tag="recip", name=f"r_{p}_{j}_{hh}")
                    nc.vector.reciprocal_approx_fast(r[:], l0[:])
                    rb = rp.tile([64, 512], f32, tag="rbcast",
                                 name=f"rb_{p}_{j}_{hh}")
                    nc.gpsimd.partition_broadcast(rb[:], r[:])
                    nc.vector.tensor_mul(
                        CT[64 * hh : 64 * hh + 64, p, 512 * j : 512 * (j + 1)],
                        ots[hh][0:HEAD_SIZE, :],
                        rb[:],
                    )
                # output projection for the t-blocks whose CT columns are
                # complete (pair 3 only)
                if p == NPAIR - 1:
                    for tb in range(4 * j, 4 * j + 4):
                        for u in proj_units(tb):
                            u()

    nc.compile()
    return nc


def _get_nc(mm_dt_name: str):
    if mm_dt_name not in _CACHED_NC:
        _CACHED_NC[mm_dt_name] = _build_bass(mm_dt_name)
    return _CACHED_NC[mm_dt_name]


def _make_trib(np_dt):
    # trib[s, t] = 1 where s <= t (allowed), 0 above the diagonal.
    s = np.arange(P)[:, None]
    t = np.arange(P)[None, :]
    return np.where(s <= t, 1.0, 0.0).astype(np_dt)


def _prep_in_maps(x, Wq, Wk, Wv, Wo, np_dt):
    trib = _make_trib(np_dt)
    in_maps = []
    for core in range(8):
        b, g = core // 2, core % 2
        hsl = slice(8 * g, 8 * (g + 1))
        xT = np.ascontiguousarray(x[b].T).astype(np_dt)
        wq = np.ascontiguousarray(
            Wq[hsl].transpose(1, 0, 2).reshape(N_EMBED, DGRP)
        ).astype(np_dt)
        wk = np.ascontiguousarray(
            Wk[hsl].transpose(1, 0, 2).reshape(N_EMBED, DGRP)
        ).astype(np_dt)
        wv = np.ascontiguousarray(
            Wv[hsl].transpose(1, 0, 2).reshape(N_EMBED, DGRP)
        ).astype(np_dt)
        wo = np.ascontiguousarray(Wo[DGRP * g : DGRP * (g + 1)]).astype(np_dt)
        in_maps.append(
            {"xT": xT, "wq": wq, "wk": wk, "wv": wv, "wo": wo, "trib": trib}
        )
    return in_maps


def run_on_hw(inputs, mm_dt_name=MM_DT, trace=False, tmpdir=None):
    """Returns (out [4, 2048, 1024] f32, BassKernelResults)."""
    from concourse.bass_utils import run_bass_kernel_spmd

    x = np.asarray(inputs["x"], dtype=np.float32)
    Wq = np.asarray(inputs["Wq"], dtype=np.float32)
    Wk = np.asarray(inputs["Wk"], dtype=np.float32)
    Wv = np.asarray(inputs["Wv"], dtype=np.float32)
    Wo = np.asarray(inputs["Wo"], dtype=np.float32)
    bo = np.asarray(inputs["bo"], dtype=np.float32)

    np_dt = ml_dtypes.bfloat16 if mm_dt_name == "bf16" else np.float32
    in_maps = _prep_in_maps(x, Wq, Wk, Wv, Wo, np_dt)
    nc = _get_nc(mm_dt_name)
    res = run_bass_kernel_spmd(
        nc, in_maps, core_ids=list(range(8)), trace=trace, tmpdir=tmpdir
    )
    out = np.empty((B, T, N_EMBED), dtype=np.float32)
    for b in range(B):
        out[b] = (res.results[2 * b]["y"].astype(np.float32)
                  + res.results[2 * b + 1]["y"].astype(np.float32) + bo)
    return out, res


def kernel(**inputs) -> np.ndarray:
    out, _ = run_on_hw(inputs)
    return out


# revision 9
# speedup vs baseline: 1.0479x; 1.0479x over previous
"""Trainium2 Bass kernel for multi-head causal attention (nn_MultiHeadAttention).

Full-model shapes: x [4, 2048, 1024], 16 heads x 64 head-size, Wo [1024, 1024].

Sharding (8 cores): shard = (batch b, head-group g of 8 heads); core = 2*b + g.
Each core computes, for its batch and its 8 heads:
  QT/KT [hs, T] (head pairs packed into 128 partitions) and VA = [V | 1] [T, 65],
  ST = K @ Q^T blocks [s-part, t-free] (causal blocks only, band narrowed),
  expST = exp(ST/8), diagonal 128x128 sub-block masked post-exp with a 0/1 tri,
  OT = [V | 1]^T @ expST  -> rows 0:64 unnormalized output (transposed),
                             row 64 the softmax denominator l(t),
  concatT = OT[0:64] * (1/l) broadcast,
  y_partial = concatT^T @ Wo[512*g : 512*(g+1)]  [T, 1024]  (stored bf16).
Host sums the two head-group partials per batch and adds the bias.

Head pairs share one [128,1024] ST psum tile (h0 -> cols 0:512, h1 -> 512:1024,
PE row groups 0:63 / 64:127) so a single strided ACTIVATE computes exp for
both heads. Softmax needs no max-subtraction: scores are q.k/8 with |q|,|k|
~ 0.6, so exp() stays in a tiny range and matches jax.nn.softmax to fp32
rounding.

Scheduling: scalar ACT (exp) is the per-chunk pacing engine (~1.1us for
[128,1024]); projection / output matmuls are drained as fine-grained filler
units (1-2 matmuls) between attention chunks so the PE stays dense without
starving ACT.  K/Q projection groups accumulate two j-tiles per weight load
so walrus dedups the LDWEIGHTS of the matmul pair.
"""

import os
from contextlib import ExitStack

import numpy as np
import ml_dtypes

N_HEADS = 16
HEAD_SIZE = 64
N_EMBED = 1024
B, T = 4, 2048
P = 128
NE = N_EMBED // P          # 8 e-chunks
NT5 = T // 512             # 4 t-tiles of 512
NT1 = T // P               # 16 t-blocks of 128
NH = N_HEADS // 2          # 8 heads per core
NPAIR = NH // 2            # 4 head pairs per core
DGRP = NH * HEAD_SIZE      # 512 concat rows per core

# matmul dtype: "bf16" or "f32r" (fp32 data, relaxed-precision PE mode)
MM_DT = os.environ.get("KERNEL_MM_DT", "bf16")

_CACHED_NC = {}


def _build_bass(mm_dt_name: str):
    import concourse.bass as bass  # noqa: F401
    import concourse.tile as tile
    from concourse import bacc, mybir

    f32 = mybir.dt.float32
    if mm_dt_name == "bf16":
        dt_mm = mybir.dt.bfloat16
        mm_cast = lambda ap: ap  # noqa: E731
    else:
        dt_mm = f32
        mm_cast = lambda ap: ap.bitcast(mybir.dt.float32r)  # noqa: E731
    Exp = mybir.ActivationFunctionType.Exp

    nc = bacc.Bacc("TRN2", target_bir_lowering=False, debug=False, num_devices=8)

    xT_d = nc.dram_tensor("xT", [N_EMBED, T], dt_mm, kind="ExternalInput")
    wq_d = nc.dram_tensor("wq", [N_EMBED, DGRP], dt_mm, kind="ExternalInput")
    wk_d = nc.dram_tensor("wk", [N_EMBED, DGRP], dt_mm, kind="ExternalInput")
    wv_d = nc.dram_tensor("wv", [N_EMBED, DGRP], dt_mm, kind="ExternalInput")
    wo_d = nc.dram_tensor("wo", [DGRP, N_EMBED], dt_mm, kind="ExternalInput")
    trib_d = nc.dram_tensor("trib", [P, P], dt_mm, kind="ExternalInput")
    y_d = nc.dram_tensor("y", [T, N_EMBED], dt_mm, kind="ExternalOutput")

    xT_ap = xT_d.ap().rearrange("(o p) t -> p o t", p=P)    # [128, 8, 2048]
    wq_ap = wq_d.ap().rearrange("(o p) m -> p o m", p=P)    # [128, 8, 512]
    wk_ap = wk_d.ap().rearrange("(o p) m -> p o m", p=P)
    wv_ap = wv_d.ap().rearrange("(o p) m -> p o m", p=P)
    wo_ap = wo_d.ap().rearrange("(o p) e -> p o e", p=P)    # [128, 4, 1024]
    y_ap = y_d.ap().rearrange("(o p) e -> p o e", p=P)      # [128, 16, 1024]

    with tile.TileContext(nc) as tc, ExitStack() as ctx:
        const = ctx.enter_context(tc.tile_pool(name="const", bufs=1))
        persist = ctx.enter_context(tc.tile_pool(name="persist", bufs=1))
        # PSUM: pb1 4x1 banks (filler psums + OT tiles) + stp 2x2 banks = 8.
        pb1 = ctx.enter_context(tc.tile_pool(name="pb1", bufs=4, space="PSUM"))
        stp = ctx.enter_context(tc.tile_pool(name="stp", bufs=2, space="PSUM"))
        expool = ctx.enter_context(tc.tile_pool(name="expool", bufs=6))
        rp = ctx.enter_context(tc.tile_pool(name="rp", bufs=4))
        ysbp = ctx.enter_context(tc.tile_pool(name="ysb", bufs=2))

        trib_sb = const.tile([P, P], dt_mm)
        warm = const.tile([1, 2], f32)

        # persistent tensors (bf16: ~125 KB/partition total incl pools)
        xt_sb = persist.tile([P, NE, T], dt_mm)
        wv_sb = persist.tile([P, NE, DGRP], dt_mm)
        wk_sb = persist.tile([P, NE, DGRP], dt_mm)
        wq_sb = persist.tile([P, NE, DGRP], dt_mm)
        wo_sb = persist.tile([P, NPAIR, N_EMBED], dt_mm)
        VA = persist.tile([P, NT1, NH, HEAD_SIZE + 1], dt_mm)
        CT = persist.tile([P, NPAIR, T], dt_mm)
        QTs = [persist.tile([P, T], dt_mm, name=f"QT_{pp}") for pp in range(NPAIR)]
        KTs = [persist.tile([P, T], dt_mm, name=f"KT_{pp}") for pp in range(NPAIR)]

        # ACT table pre-warm: first exp pays the ~2.7us table load during the
        # initial DMA wait instead of on the first attention chunk.
        nc.vector.memset(warm[:], 0.0)
        nc.scalar.activation(warm[:], warm[:], Exp, scale=1.0)
        nc.vector.memset(VA[:, :, :, HEAD_SIZE : HEAD_SIZE + 1], 1.0)

        # ---- input DMAs, consumption order, large tensors split so the
        # pieces land on parallel queues and the first matmuls start early.
        # Head phase needs: xt (t 0:512 for all e), wv (all), wk/wq pair-0
        # columns.
        nc.sync.dma_start(trib_sb[:], trib_d.ap())
        for e in range(NE):
            nc.sync.dma_start(xt_sb[:, e, 0:512], xT_ap[:, e, 0:512])
            nc.sync.dma_start(wv_sb[:, e, :], wv_ap[:, e, :])
        for h in range(2):
            nc.sync.dma_start(wk_sb[:, 4 * h : 4 * h + 4, 0:P],
                              wk_ap[:, 4 * h : 4 * h + 4, 0:P])
            nc.sync.dma_start(wq_sb[:, 4 * h : 4 * h + 4, 0:P],
                              wq_ap[:, 4 * h : 4 * h + 4, 0:P])
        for tq in range(1, 4):
            for e in range(NE):
                nc.sync.dma_start(xt_sb[:, e, 512 * tq : 512 * (tq + 1)],
                                  xT_ap[:, e, 512 * tq : 512 * (tq + 1)])
        for h in range(2):
            nc.sync.dma_start(wk_sb[:, 4 * h : 4 * h + 4, P:DGRP],
                              wk_ap[:, 4 * h : 4 * h + 4, P:DGRP])
            nc.sync.dma_start(wq_sb[:, 4 * h : 4 * h + 4, P:DGRP],
                              wq_ap[:, 4 * h : 4 * h + 4, P:DGRP])
        for dc in range(NPAIR):
            nc.sync.dma_start(wo_sb[:, dc, :], wo_ap[:, dc, :])

        # ---------------- V projection (one t-block of 128) ----------------
        # stationary = xt chunk, moving = wv; out [t 128, 512] -> VA[:,tb,:,1:]
        def v_units(tb):
            hold = {}

            def mm(e):
                if e == 0:
                    hold["vp"] = pb1.tile([P, DGRP], f32, tag="b1",
                                          name=f"v_ps_{tb}")
                nc.tensor.matmul(
                    hold["vp"][:],
                    mm_cast(xt_sb[:, e, P * tb : P * (tb + 1)]),
                    mm_cast(wv_sb[:, e, :]),
                    start=(e == 0),
                    stop=(e == NE - 1),
                )

            def evict():
                nc.vector.tensor_copy(
                    VA[:, tb, :, 0:HEAD_SIZE],
                    hold["vp"][:].rearrange("p (h d) -> p h d", d=HEAD_SIZE),
                )

            return [lambda e=e: mm(e) for e in range(NE)] + [evict]

        # -------- K/Q projection: two j-tiles per stationary load ---------
        # stationary = w chunk [e 128, pair 128]; for each e the two matmuls
        # (j = 2jj, 2jj+1) share the stationary so walrus dedups the
        # LDWEIGHTS.  Two psum tiles held across the e loop.
        def qk_units(p, which, jj):
            w_sb = wk_sb if which == 0 else wq_sb
            dst = KTs[p] if which == 0 else QTs[p]
            hold = {}

            def mm2(e):
                if e == 0:
                    for ji in range(2):
                        hold[ji] = pb1.tile([P, 512], f32, tag="b1",
                                            name=f"qk_ps_{p}_{which}_{jj}_{ji}")
                for ji in range(2):
                    j = 2 * jj + ji
                    nc.tensor.matmul(
                        hold[ji][:],
                        mm_cast(w_sb[:, e, P * p : P * (p + 1)]),
                        mm_cast(xt_sb[:, e, 512 * j : 512 * (j + 1)]),
                        start=(e == 0),
                        stop=(e == NE - 1),
                    )

            def evict(ji):
                j = 2 * jj + ji
                nc.vector.tensor_copy(dst[:, 512 * j : 512 * (j + 1)],
                                      hold[ji][:])

            return ([lambda e=e: mm2(e) for e in range(NE)]
                    + [lambda ji=ji: evict(ji) for ji in range(2)])

        # ---- output projection for one t-block: y[tb] = CT^T @ Wo-half ----
        # (dc, eh) loop: the eh pair shares the CT stationary (LDW dedup).
        def proj_units(tb):
            hold = {}

            def mm2(dc):
                if dc == 0:
                    hold[0] = pb1.tile([P, 512], f32, tag="b1",
                                       name=f"y_ps_{tb}_0")
                    hold[1] = pb1.tile([P, 512], f32, tag="b1",
                                       name=f"y_ps_{tb}_1")
                for eh in range(2):
                    nc.tensor.matmul(
                        hold[eh][:],
                        mm_cast(CT[:, dc, P * tb : P * (tb + 1)]),
                        mm_cast(wo_sb[:, dc, 512 * eh : 512 * (eh + 1)]),
                        start=(dc == 0),
                        stop=(dc == NPAIR - 1),
                    )

            def evict():
                ysb = ysbp.tile([P, N_EMBED], dt_mm, tag="ysb", name=f"ysb_{tb}")
                nc.vector.tensor_copy(ysb[:, 0:512], hold[0][:])
                nc.vector.tensor_copy(ysb[:, 512:1024], hold[1][:])
                for q in range(2):
                    nc.sync.dma_start(y_ap[:, tb, 512 * q : 512 * (q + 1)],
                                      ysb[:, 512 * q : 512 * (q + 1)])

            return [lambda dc=dc: mm2(dc) for dc in range(NPAIR)] + [evict]

        # -------- head phase: V tb0-3, K/Q jj=0 for pairs 0 and 1 --------
        for tb in range(4):
            for u in v_units(tb):
                u()
        for which in range(2):
            for u in qk_units(0, which, 0):
                u()
        for which in range(2):
            for u in qk_units(1, which, 0):
                u()

        # ---- filler queue: flat list of units with slot deadlines ----
        # Attention iterates j-outer, p-inner; slot key = 4*j + p.  A unit
        # must have run by the END of its deadline slot.  Queue order
        # respects readiness (y-proj tb group becomes ready at slot 4j+3).
        fill_units = []

        def add_group(units, deadline):
            for u in units:
                fill_units.append((deadline, u))

        add_group(qk_units(2, 0, 0), 1)   # needed by slot (j0, p2)
        add_group(qk_units(2, 1, 0), 1)
        add_group(qk_units(3, 0, 0), 2)   # (j0, p3)
        add_group(qk_units(3, 1, 0), 2)
        for tb in range(4, 8):            # VA tb4-7: (j1, p0) chunks c>=4
            add_group(v_units(tb), 3)
        add_group(qk_units(0, 0, 1), 6)   # jj1: hoist at end of (j1, p3)
        add_group(qk_units(0, 1, 1), 6)   #   reads (j2, p0) c=0,1
        for tb in range(8, 12):           # VA tb8-11: (j2, p0) c>=8
            add_group(v_units(tb), 7)
        add_group(qk_units(1, 0, 1), 7)
        add_group(qk_units(1, 1, 1), 7)
        add_group(qk_units(2, 0, 1), 8)
        add_group(qk_units(2, 1, 1), 8)
        add_group(qk_units(3, 0, 1), 9)
        add_group(qk_units(3, 1, 1), 9)
        for tb in range(12, 16):          # VA tb12-15: (j3, p0) c>=12
            add_group(v_units(tb), 11)
        for tb in range(0, 4):            # y tb0-3: CT ready after slot 3
            add_group(proj_units(tb), 11)
        for tb in range(4, 8):            # ready after slot 7
            add_group(proj_units(tb), 15)
        for tb in range(8, 12):           # ready after slot 11
            add_group(proj_units(tb), 15)
        # y tb12-15 runs in the tail after the last attention slot.

        fill_pos = [0]

        def drain_to(target):
            while fill_pos[0] < min(target, len(fill_units)):
                fill_units[fill_pos[0]][1]()
                fill_pos[0] += 1

        def deadline_target(key):
            # index just past the last unit with deadline <= key
            t = fill_pos[0]
            for i in range(fill_pos[0], len(fill_units)):
                if fill_units[i][0] <= key:
                    t = i + 1
            return t

        # ------- attention -------
        pre_es = {}

        def st_exp(p, j, c):
            KTp, QTp = KTs[p], QTs[p]
            off = P * max(0, c - 4 * j)
            stq = stp.tile([P, 1024], f32, tag="st", name=f"st_{p}_{j}_{c}")
            for hh in range(2):
                nc.tensor.matmul(
                    stq[:, 512 * hh + off : 512 * hh + 512],
                    mm_cast(KTp[64 * hh : 64 * hh + 64, P * c : P * (c + 1)]),
                    mm_cast(
                        QTp[64 * hh : 64 * hh + 64,
                            512 * j + off : 512 * (j + 1)]
                    ),
                    start=True,
                    stop=True,
                )
            stv = stq[:].rearrange("p (g t) -> p g t", g=2)
            es = expool.tile([P, 1024], dt_mm, tag="es",
                             name=f"es_{p}_{j}_{c}")
            esv = es[:].rearrange("p (g t) -> p g t", g=2)
            nc.scalar.activation(
                esv[:, :, off:512], stv[:, :, off:512], Exp, scale=0.125
            )
            if c >= 4 * j:  # diagonal sub-block: zero the upper triangle
                dv = esv[:, :, off : off + P]
                nc.vector.tensor_mul(
                    dv, dv, trib_sb[:, None, :].to_broadcast((P, 2, P))
                )
            return es

        global_chunk = [0]
        TOTAL_CHUNKS = sum(4 * j + 4 for j in range(NT5)) * NPAIR  # 160
        slots = [(j, p) for j in range(NT5) for p in range(NPAIR)]

        for si, (j, p) in enumerate(slots):
            key = si
            if True:
                ots = [
                    pb1.tile([HEAD_SIZE + 1, 512], f32, tag="b1",
                             name=f"ot_{p}_{j}_{hh}")
                    for hh in range(2)
                ]
                ncs = 4 * j + 4
                # pacing: drain the remaining queue evenly over the
                # remaining chunks, clamped so this slot's deadline units
                # finish by its last chunk
                dl_target = deadline_target(key)
                for c in range(ncs):
                    off = P * max(0, c - 4 * j)   # band narrowing
                    if (p, j, c) in pre_es:
                        es = pre_es.pop((p, j, c))
                    else:
                        es = st_exp(p, j, c)
                    if c == ncs - 1:
                        # hoist the next slot's first two ST+exp ahead of
                        # the last PVs so ACT is never starved across the
                        # boundary
                        if si + 1 < len(slots):
                            nj, npp = slots[si + 1]
                            pre_es[(npp, nj, 0)] = st_exp(npp, nj, 0)
                            pre_es[(npp, nj, 1)] = st_exp(npp, nj, 1)
                    for hh in range(2):
                        nc.tensor.matmul(
                            ots[hh][:, off:512],
                            mm_cast(VA[:, c, 2 * p + hh, :]),
                            mm_cast(es[:, 512 * hh + off : 512 * hh + 512]),
                            start=(c == 0),
                            stop=(c == ncs - 1),
                        )
                    global_chunk[0] += 1
                    rem_chunks = TOTAL_CHUNKS - global_chunk[0] + 1
                    rem_units = len(fill_units) - fill_pos[0]
                    tgt = fill_pos[0] + (rem_units + rem_chunks - 1) // rem_chunks
                    need = dl_target - fill_pos[0]
                    if need > 0:
                        tgt = max(tgt, fill_pos[0]
                                  + (need * (c + 1) + ncs - 1) // ncs)
                    drain_to(tgt)
                for hh in range(2):
                    # OT rows 0:64 unnormalized output, row 64 is l(t).
                    # l staged to partition 0: reciprocal_approx_fast reads
                    # garbage when its input AP starts at partition 64.
                    l0 = rp.tile([1, 512], f32, tag="lrow", name=f"l0_{p}_{j}_{hh}")
                    nc.vector.tensor_copy(l0[:], ots[hh][64:65, :])
                    r = rp.tile([1, 512], f32, tag="recip", name=f"r_{p}_{j}_{hh}")
                    nc.vector.reciprocal_approx_fast(r[:], l0[:])
                    rb = rp.tile([64, 512], f32, tag="rbcast",
                                 name=f"rb_{p}_{j}_{hh}")
                    nc.gpsimd.partition_broadcast(rb[:], r[:])
                    nc.vector.tensor_mul(
                        CT[64 * hh : 64 * hh + 64, p, 512 * j : 512 * (j + 1)],
                        ots[hh][0:HEAD_SIZE, :],
                        rb[:],
                    )

        # tail: drain any queue leftovers, then the last four y t-blocks
        drain_to(len(fill_units))
        for tb in range(12, 16):
            for u in proj_units(tb):
                u()

    nc.compile()
    return nc


def _get_nc(mm_dt_name: str):
    if mm_dt_name not in _CACHED_NC:
        _CACHED_NC[mm_dt_name] = _build_bass(mm_dt_name)
    return _CACHED_NC[mm_dt_name]


def _make_trib(np_dt):
    # trib[s, t] = 1 where s <= t (allowed), 0 above the diagonal.
    s = np.arange(P)[:, None]
    t = np.arange(P)[None, :]
    return np.where(s <= t, 1.0, 0.0).astype(np_dt)


def _prep_in_maps(x, Wq, Wk, Wv, Wo, np_dt):
    trib = _make_trib(np_dt)
    in_maps = []
    for core in range(8):
        b, g = core // 2, core % 2
        hsl = slice(8 * g, 8 * (g + 1))
        xT = np.ascontiguousarray(x[b].T).astype(np_dt)
        wq = np.ascontiguousarray(
            Wq[hsl].transpose(1, 0, 2).reshape(N_EMBED, DGRP)
        ).astype(np_dt)
        wk = np.ascontiguousarray(
            Wk[hsl].transpose(1, 0, 2).reshape(N_EMBED, DGRP)
        ).astype(np_dt)
        wv = np.ascontiguousarray(
            Wv[hsl].transpose(1, 0, 2).reshape(N_EMBED, DGRP)
        ).astype(np_dt)
        wo = np.ascontiguousarray(Wo[DGRP * g : DGRP * (g + 1)]).astype(np_dt)
        in_maps.append(
            {"xT": xT, "wq": wq, "wk": wk, "wv": wv, "wo": wo, "trib": trib}
        )
    return in_maps


def run_on_hw(inputs, mm_dt_name=MM_DT, trace=False, tmpdir=None):
    """Returns (out [4, 2048, 1024] f32, BassKernelResults)."""
    from concourse.bass_utils import run_bass_kernel_spmd

    x = np.asarray(inputs["x"], dtype=np.float32)
    Wq = np.asarray(inputs["Wq"], dtype=np.float32)
    Wk = np.asarray(inputs["Wk"], dtype=np.float32)
    Wv = np.asarray(inputs["Wv"], dtype=np.float32)
    Wo = np.asarray(inputs["Wo"], dtype=np.float32)
    bo = np.asarray(inputs["bo"], dtype=np.float32)

    np_dt = ml_dtypes.bfloat16 if mm_dt_name == "bf16" else np.float32
    in_maps = _prep_in_maps(x, Wq, Wk, Wv, Wo, np_dt)
    nc = _get_nc(mm_dt_name)
    res = run_bass_kernel_spmd(
        nc, in_maps, core_ids=list(range(8)), trace=trace, tmpdir=tmpdir
    )
    out = np.empty((B, T, N_EMBED), dtype=np.float32)
    for b in range(B):
        out[b] = (res.results[2 * b]["y"].astype(np.float32)
                  + res.results[2 * b + 1]["y"].astype(np.float32) + bo)
    return out, res


def kernel(**inputs) -> np.ndarray:
    out, _ = run_on_hw(inputs)
    return out
